# revision 36
# baseline (speedup 1.0000x reference)
"""GNN mean-aggregation message passing on 8 Trainium2 NeuronCores.

out[t] = mean_{e: tgt[e]==t} x[src[e]]   (0 if no incoming edges)

Strategy (target-sharded, uniform SPMD schedule):
  - Targets are dealt to cores serpentine-by-degree so per-(core,group) edge
    counts are balanced; each core owns 12544 output slots (98 groups of 128).
  - Host packs x as bf16 hi|lo pairs -> [N_pad, 128] bf16 (256B rows). The
    hi+lo split recovers ~fp32 precision after the f32 PSUM accumulation.
  - Edges are routed to the owning core, ordered by (supergroup, src-chunk,
    target) and packed into 128-edge slots. Every slot is bound (at compile
    time, uniformly across cores) to a target group g; its edges may only
    reference groups {g, g+1}. Two matmuls per slot (one-hot selection
    matrices vs. iota 0..127 / 128..255) accumulate into per-group PSUM.
  - Sources are gathered straight from HBM with dma_gather (int16 indices,
    4 chunks of 25088 rows to fit the int16 range). The SWDGE descriptor
    emission (~8ns/idx) is the wall; the schedule minimizes total idx count
    and keeps the last supergroup tiny so the matmul tail is short.
  - Finalize per group: (psum_hi + psum_lo) * inv_count -> DMA out.
"""
import sys

sys.path.insert(0, "/opt/trn_rl_repo")

import numpy as np
import ml_dtypes

bf16 = ml_dtypes.bfloat16

# ---- problem constants (hardcoded per harness contract) ----
N, F, E = 100000, 64, 1600000
P = 128
NCORES = 8
TPC = 12544                  # target slots per core (= 98 * 128)
GPC = TPC // P               # 98 groups per core
NCHUNKS = 4
CHUNK = 25088                # source rows per chunk (< 32768 for int16 idx)
NPAD = NCHUNKS * CHUNK       # 100352 padded source rows
SG_SIZES = [26, 26, 26, 19, 1]   # supergroups; tiny last one -> short tail
ELEM = 2 * F                 # 128 bf16 per packed row = 256B
SENT = 384.0                 # sentinel row id for pad edges (no iota match)
SEL_PIECE = 16               # slots per one-hot compare instruction
GCALL = 56                   # slots per dma_gather call (ring-wrap above ~8K idx)
NQUEUES = 4                  # SWDGE queues; queue q emits on Q7 core pair {2q,2q+1}
SINGLE_PACKET = False        # True (>64-desc packets) faults the DMA engines
DMA_SCRATCH = 16384          # SWDGE ring carveout bytes/partition (deeper didn't help)


def _chunk_order(sched, s):
    return sorted(range(NCHUNKS), key=lambda c: (-len(sched[s][c]), c))


def _call_sizes(w):
    """Split a bin of w slots into equal-ish gather calls of <=GCALL slots."""
    ncalls = -(-w // GCALL)
    base, rem = divmod(w, ncalls)
    return [base + 1] * rem + [base] * (ncalls - rem)


def _balance_targets(tgt):
    """Serpentine-by-degree target -> (core, rank) assignment.

    Returns perm_o2n[old_target] = new_target_id (core*TPC + rank)."""
    deg = np.bincount(tgt, minlength=N)
    order = np.argsort(-deg, kind="stable")
    perm = np.empty(N, np.int64)
    pos = np.arange(N)
    rows = pos // NCORES
    cols = pos % NCORES
    cores = np.where(rows % 2 == 0, cols, NCORES - 1 - cols)
    for k in range(NCORES):
        ids = order[cores == k]
        perm[ids] = k * TPC + np.arange(ids.size)
    return perm


def _host_prep(x, edge_idx):
    """Build per-core device arrays and the shared slot schedule."""
    x = np.asarray(x, np.float32)
    src = np.asarray(edge_idx[0], np.int64)
    tgt_orig = np.asarray(edge_idx[1], np.int64)

    perm = _balance_targets(tgt_orig)
    tgt = perm[tgt_orig]

    # packed hi|lo bf16 table
    hi = x.astype(bf16)
    lo = (x - hi.astype(np.float32)).astype(bf16)
    xp = np.zeros((NPAD, ELEM), bf16)
    xp[:N, :F] = hi
    xp[:N, F:] = lo

    inv_cnt = np.bincount(tgt, minlength=NCORES * TPC).astype(np.float32)
    inv_cnt = 1.0 / np.maximum(inv_cnt, 1.0)

    core = tgt // TPC
    chunk = src // CHUNK
    gl = (tgt // P) - core * GPC          # local group 0..97
    sg_bounds = np.cumsum([0] + SG_SIZES)
    sg = np.searchsorted(sg_bounds, gl, side="right") - 1
    order = np.lexsort((tgt, chunk, sg, core))

    nsg = len(SG_SIZES)
    bin_id = (core * nsg + sg) * NCHUNKS + chunk
    bin_sizes = np.bincount(bin_id, minlength=NCORES * nsg * NCHUNKS)
    bin_starts = np.zeros(NCORES * nsg * NCHUNKS + 1, np.int64)
    np.cumsum(bin_sizes, out=bin_starts[1:])

    gl_sorted = gl[order]
    tgt_sorted = tgt[order]
    src_sorted = src[order]
    chunk_sorted = chunk[order]

    # ---- build shared schedule: per (sg, c) the block label list ----
    sched = []          # sched[sg][c] = np.array of block labels h (bin-local)
    for s in range(nsg):
        gs = SG_SIZES[s]
        row = []
        for c in range(NCHUNKS):
            e_kh = np.zeros((NCORES, gs), np.int64)
            for k in range(NCORES):
                b = (k * nsg + s) * NCHUNKS + c
                seg = gl_sorted[bin_starts[b]:bin_starts[b + 1]] - sg_bounds[s]
                if seg.size:
                    e_kh[k] = np.bincount(seg, minlength=gs)
            labels = []
            r = e_kh[:, 0].astype(np.int64)
            for h in range(gs):
                s_h = int(np.ceil(r / P).max())
                labels.extend([h] * s_h)
                cap = s_h * P - r
                if h + 1 < gs:
                    r = np.maximum(0, e_kh[:, h + 1] - cap)
                else:
                    assert (cap >= 0).all()
            row.append(np.asarray(labels, np.int64))
        sched.append(row)

    tot_slots = sum(len(row[c]) for row in sched for c in range(NCHUNKS))
    tot = tot_slots * P

    # ---- per-core edge placement into the uniform slot stream ----
    src_local = np.zeros((NCORES, tot), np.int16)
    trow = np.full((NCORES, tot), SENT, np.float32)
    spill = np.zeros(tot_slots, bool)   # slot has any lane in group h+1
    for k in range(NCORES):
        base = 0
        for s in range(nsg):
            for c in _chunk_order(sched, s):
                labels = sched[s][c]
                b = (k * nsg + s) * NCHUNKS + c
                lo_i, hi_i = bin_starts[b], bin_starts[b + 1]
                garr = gl_sorted[lo_i:hi_i] - sg_bounds[s]
                p = 0
                for bi, h in enumerate(labels):
                    upper = np.searchsorted(garr, h + 1, side="right")
                    take = min(P, upper - p)
                    if take > 0:
                        sl = slice(lo_i + p, lo_i + p + take)
                        pos = base + bi * P
                        src_local[k, pos:pos + take] = (
                            src_sorted[sl] - chunk_sorted[sl] * CHUNK
                        ).astype(np.int16)
                        trow[k, pos:pos + take] = (
                            tgt_sorted[sl] % P + P * (garr[p:p + take] - h)
                        ).astype(np.float32)
                        if (garr[p:p + take] > h).any():
                            spill[base // P + bi] = True
                        p += take
                assert p == hi_i - lo_i, (
                    f"core {k} sg {s} c {c}: placed {p} of {hi_i - lo_i}"
                )
                base += len(labels) * P
        assert base == tot

    # device layouts
    idx_dev = [
        np.tile(src_local[k].reshape(tot // 16, 16).T, (8, 1)).copy()
        for k in range(NCORES)
    ]
    # split trow into main (tgt%128 within slot's own group) and spill
    # (compact columns, one per spill slot) so sel compares are 128-wide
    spill_slots = np.flatnonzero(spill)               # shared across cores
    n_spill = len(spill_slots)
    nsp_pad = (n_spill + SEL_PIECE - 1) // SEL_PIECE * SEL_PIECE
    trow_dev = []
    trowsp_dev = []
    for k in range(NCORES):
        v = trow[k].reshape(tot // P, P)              # [slot, lane] 0..255/SENT
        main = np.where(v < P, v, SENT).T.astype(bf16).copy()
        sp = np.full((nsp_pad, P), SENT, np.float32)
        vs = v[spill_slots]
        sp[:n_spill] = np.where((vs >= P) & (vs < 2 * P), vs - P, SENT)
        trow_dev.append(main)
        trowsp_dev.append(sp.T.astype(bf16).copy())
    invc_dev = [
        inv_cnt[k * TPC:(k + 1) * TPC].reshape(GPC, P).T.copy()
        for k in range(NCORES)
    ]
    iota_dev = np.tile(np.arange(P, dtype=np.float32), (P, SEL_PIECE)).astype(bf16)
    return (xp, idx_dev, trow_dev, trowsp_dev, invc_dev, iota_dev, sched, tot,
            spill, perm)


def _build_program(sched, tot, spill):
    from concourse import bacc, mybir, tile

    nsg = len(SG_SIZES)
    gsg_max = max(SG_SIZES)
    spill_cols = np.cumsum(spill) - spill      # slot -> its trowsp column
    n_spill = int(spill.sum())
    nsp_pad = (n_spill + SEL_PIECE - 1) // SEL_PIECE * SEL_PIECE

    nc = bacc.Bacc(None, target_bir_lowering=False, debug=False,
                   num_swdge_queues=NQUEUES,
                   dynamic_dma_scratch_size=DMA_SCRATCH)
    t_x = nc.dram_tensor("xp", [NPAD, ELEM], mybir.dt.bfloat16, kind="ExternalInput")
    t_idx = nc.dram_tensor("idx", [P, tot // 16], mybir.dt.int16, kind="ExternalInput")
    t_trow = nc.dram_tensor("trow", [P, tot // P], mybir.dt.bfloat16, kind="ExternalInput")
    t_trowsp = nc.dram_tensor("trowsp", [P, nsp_pad], mybir.dt.bfloat16, kind="ExternalInput")
    t_invc = nc.dram_tensor("invc", [P, GPC], mybir.dt.float32, kind="ExternalInput")
    t_iota = nc.dram_tensor("iota", [P, SEL_PIECE * P], mybir.dt.bfloat16, kind="ExternalInput")
    # out laid out [partition, group*F] so the finalize DMA is one contiguous
    # run per partition (16 big descriptors/call instead of 128*gs small ones)
    t_out = nc.dram_tensor("out", [P, GPC * F], mybir.dt.float32, kind="ExternalOutput")

    with tile.TileContext(nc) as tc:
        with (
            tc.tile_pool(name="const", bufs=1) as cpool,
            tc.tile_pool(name="msgs", bufs=4) as mpool,
            tc.tile_pool(name="sel", bufs=3) as spool,
            tc.tile_pool(name="spsel", bufs=2) as sppool,
            tc.tile_pool(name="stage", bufs=2) as stpool,
            tc.tile_pool(name="psum", bufs=8, space="PSUM") as ppool,
        ):
            # idx split into a first-call tile and a rest tile: a single tile
            # would give a false WAR dep (rest-load waits on gather 0's read,
            # gather 1 waits on the whole rest-load -> 100us serial startup)
            w_first = _call_sizes(len(sched[0][_chunk_order(sched, 0)[0]]))[0]
            idxa_t = cpool.tile([P, w_first * 8], mybir.dt.int16)
            idxb_t = cpool.tile([P, tot // 16 - w_first * 8], mybir.dt.int16)
            trow_t = cpool.tile([P, tot // P], mybir.dt.bfloat16)
            trowsp_t = cpool.tile([P, nsp_pad], mybir.dt.bfloat16)
            invc_t = cpool.tile([P, GPC], mybir.dt.float32)
            # iota pre-tiled [P, SEL_PIECE, P] so the compare's in1 is a plain
            # contiguous read (no stride-0 broadcast on that operand)
            iota_b = cpool.tile([P, SEL_PIECE, P], mybir.dt.bfloat16)

            def idx_ap(c0, c1):
                """int16 idx columns [c0, c1) across the two idx tiles."""
                if c1 <= w_first * 8:
                    return idxa_t[:, c0:c1]
                assert c0 >= w_first * 8
                return idxb_t[:, c0 - w_first * 8:c1 - w_first * 8]

            nc.sync.dma_start(
                out=iota_b[:].rearrange("r s c -> r (s c)"), in_=t_iota[:]
            )   # primes cold DMA queue
            nc.sync.dma_start(out=idxa_t[:], in_=t_idx[:, :w_first * 8])
            nc.sync.dma_start(out=idxb_t[:], in_=t_idx[:, w_first * 8:])
            nc.sync.dma_start(out=trow_t[:], in_=t_trow[:])
            nc.sync.dma_start(out=trowsp_t[:], in_=t_trowsp[:])
            nc.sync.dma_start(out=invc_t[:], in_=t_invc[:])
            qctr = [0]

            slot_off = 0     # global slot offset in the stream
            g_base = 0       # global group offset
            sp_state = [None, 0, 0]   # [tile, base_col, width] rolling spill sel
            for s in range(nsg):
                gs = SG_SIZES[s]
                nslots_psum = gs + 1
                nbanks = (nslots_psum + 3) // 4
                pts = [
                    ppool.tile([P, 4 * P], mybir.dt.float32, name=f"ps{s}_{b}", tag="ps")
                    for b in range(nbanks)
                ]
                for pt in pts:
                    nc.vector.memset(pt[:], 0.0)

                def pslot(j):
                    return pts[j // 4][:, (j % 4) * P:(j % 4 + 1) * P]

                for c in _chunk_order(sched, s):
                    labels = sched[s][c]
                    w = len(labels)
                    if w == 0:
                        continue
                    msgs_t = mpool.tile([P, w, ELEM], mybir.dt.bfloat16, name="msgs")
                    # split bins into EQUAL <=GCALL-slot gather calls (equal
                    # sizes keep the ~4-deep Pool dispatch window in lockstep;
                    # much above ~8K idx/call hits SWDGE ring-wrap stalls).
                    # Round-robin the 4 SWDGE queues so descriptor emission
                    # runs on all 4 Q7 core pairs concurrently.
                    g0 = 0
                    for gw in _call_sizes(w):
                        nc.gpsimd.dma_gather(
                            out_ap=msgs_t[:, g0:g0 + gw, :],
                            in_ap=t_x[c * CHUNK:(c + 1) * CHUNK, :],
                            idxs_ap=idx_ap((slot_off + g0) * 8,
                                           (slot_off + g0 + gw) * 8),
                            num_idxs=gw * P,
                            num_idxs_reg=gw * P,
                            elem_size=ELEM,
                            single_packet=SINGLE_PACKET,
                            queue_num=qctr[0] % NQUEUES,
                        )
                        qctr[0] += 1
                        g0 += gw
                    sidx0 = slot_off
                    for p0 in range(0, w, SEL_PIECE):
                        pw = min(SEL_PIECE, w - p0)
                        sel_t = spool.tile([P, SEL_PIECE, P], mybir.dt.float8e4, name="sel")
                        nc.vector.tensor_tensor(
                            out=sel_t[:, :pw, :],
                            in0=trow_t[:, slot_off + p0:slot_off + p0 + pw]
                            .to_broadcast([P, pw, P]),
                            in1=iota_b[:, :pw, :],
                            op=mybir.AluOpType.is_equal,
                        )
                        for si in range(pw):
                            h = int(labels[p0 + si])
                            nc.tensor.matmul(
                                pslot(h),
                                lhsT=sel_t[:, si, :],
                                rhs=msgs_t[:, p0 + si, :],
                                start=False,
                                stop=False,
                                skip_group_check=True,
                            )
                            if spill[sidx0 + p0 + si]:
                                col = int(spill_cols[sidx0 + p0 + si])
                                if (sp_state[0] is None
                                        or col >= sp_state[1] + sp_state[2]):
                                    take = min(SEL_PIECE, nsp_pad - col)
                                    sp_t = sppool.tile(
                                        [P, SEL_PIECE, P], mybir.dt.float8e4,
                                        name="spsel")
                                    nc.vector.tensor_tensor(
                                        out=sp_t[:, :take, :],
                                        in0=trowsp_t[:, col:col + take]
                                        .to_broadcast([P, take, P]),
                                        in1=iota_b[:, :take, :],
                                        op=mybir.AluOpType.is_equal,
                                    )
                                    sp_state[:] = [sp_t, col, take]
                                nc.tensor.matmul(
                                    pslot(h + 1),
                                    lhsT=sp_state[0][:, col - sp_state[1], :],
                                    rhs=msgs_t[:, p0 + si, :],
                                    start=False,
                                    stop=False,
                                    skip_group_check=True,
                                )
                    slot_off += w

                stage_t = stpool.tile([P, gsg_max, F], mybir.dt.float32, name="stage")
                for j in range(gs):
                    tmp_t = stpool.tile([P, F], mybir.dt.float32, name="tmp", tag="tmp")
                    nc.vector.tensor_copy(out=tmp_t[:], in_=pslot(j)[:, 0:F])
                    nc.vector.tensor_add(
                        out=stage_t[:, j, :],
                        in0=tmp_t[:],
                        in1=pslot(j)[:, F:2 * F],
                    )
                    nc.vector.tensor_tensor(
                        out=stage_t[:, j, :],
                        in0=stage_t[:, j, :],
                        in1=invc_t[:, g_base + j, None].to_broadcast([P, F]),
                        op=mybir.AluOpType.mult,
                    )
                nc.sync.dma_start(
                    out=t_out[:, g_base * F:(g_base + gs) * F],
                    in_=stage_t[:].rearrange("r g f -> r (g f)")[:, :gs * F],
                )
                g_base += gs

    nc.compile()
    return nc


def kernel(x, edge_idx):
    from concourse.bass_utils import run_bass_kernel_spmd

    (xp, idx_dev, trow_dev, trowsp_dev, invc_dev, iota_dev, sched, tot, spill,
     perm) = _host_prep(x, edge_idx)
    nc = _build_program(sched, tot, spill)
    in_maps = [
        {"xp": xp, "idx": idx_dev[k], "trow": trow_dev[k],
         "trowsp": trowsp_dev[k], "invc": invc_dev[k], "iota": iota_dev}
        for k in range(NCORES)
    ]
    res = run_bass_kernel_spmd(nc, in_maps, list(range(NCORES)))
    return _gather_out(res)[perm]


def _gather_out(res):
    """[P, GPC*F] per-core device layout -> full [NCORES*TPC, F]."""
    return np.concatenate(
        [
            res.results[k]["out"]
            .reshape(P, GPC, F)
            .transpose(1, 0, 2)
            .reshape(TPC, F)
            for k in range(NCORES)
        ],
        axis=0,
    )

